# revision 25
# baseline (speedup 1.0000x reference)
"""GQA attention kernel for Trainium2, sharded over 8 NeuronCores.

Problem (hardcoded): B=4, S=1024, HID=2048, 16 query heads, 4 KV heads,
head_dim=128, RoPE (base 10000), causal softmax, O-projection.

Sharding: core c handles (batch b = c//2, head-half = c%2): 8 query heads,
2 KV heads, and the matching column/row shards of Wq/Wk/Wv/Wo. Each core
produces a partial O-projection output [S, HID]; the host sums the two
halves per batch element.

v2: all-bf16 datapath (PE runs 1 cycle/row with FWL weight loads, DMA bytes
halved), host-side transpose/relayout of x and weights (no on-device
transposes), ragged block-causal score/ctx blocks at 128-row granularity
(25% less attention work than 512-wide blocks), and a software-pipelined
per-head schedule (Q-proj of head h+1 issued between scores(h) and ctx(h))
so the tensor engine never idles long enough for HAM to re-throttle it.

Per-core layouts (partition dim first, 128 everywhere):
  xT   [128, kt(16), s(1024)]   xT[p,kt,s] = x[s, 128kt+p]          bf16
  wq   [128, h(8), kt(16), d(128)]                                  bf16
  wk/wv[128, kt(16), c(2)*d(128)]                                   bf16
  wo   [128, h(8), e(2048)]     wo[p,h,e] = Wo[1024*half+128h+p, e] bf16
  qT/kT[128d, s]   roped in T orientation via P64 perm matmul
  v    [128s, st(8)*c(2)*d(128)]  natural, for ctx stationary
  eS   [128k, ragged q spans]   exp(scores^T) bf16
  ctxT [128d, s] per head       normalized context, feeds O-proj
"""
import math
from contextlib import ExitStack

import numpy as np
import ml_dtypes

import concourse.bass as bass
import concourse.bacc as bacc
import concourse.tile as tile
from concourse import mybir
from concourse.bass_utils import run_bass_kernel_spmd

F32 = mybir.dt.float32
F32R = mybir.dt.float32r
BF16 = mybir.dt.bfloat16

B, S, HID = 4, 1024, 2048
NH, NKV, D = 16, 4, 128
HPC = 8          # query heads per core
KVPC = 2         # kv heads per core
SCALE = 1.0 / math.sqrt(D)
NKT = HID // 128  # 16 contraction tiles
NST = S // 128    # 8 sequence tiles
KVD = KVPC * D    # 256

# score block spans: for k-tile kt, valid q span is [128*kt, S), split into
# PSUM-bank-sized pieces aligned to 512 boundaries.
def _pieces(kt):
    qlo = 128 * kt
    out = []
    while qlo < S:
        qhi = min(S, (qlo // 512 + 1) * 512)
        out.append((qlo, qhi))
        qlo = qhi
    return out

# eS packing offset per kt (ragged)
ES_OFF = []
_o = 0
for _kt in range(NST):
    ES_OFF.append(_o)
    _o += S - 128 * _kt
ES_W = _o  # 4608


def build_kernel():
    nc = bacc.Bacc(None)
    xT = nc.dram_tensor("xT", [128, NKT * S], BF16, kind="ExternalInput")
    wq = nc.dram_tensor("wq", [128, HPC * NKT * D], BF16, kind="ExternalInput")
    wk = nc.dram_tensor("wk", [128, NKT * KVD], BF16, kind="ExternalInput")
    wv = nc.dram_tensor("wv", [128, NKT * KVD], BF16, kind="ExternalInput")
    wo = nc.dram_tensor("wo", [128, HPC * HID], BF16, kind="ExternalInput")
    # tables bundled: cosT(1024) | sinTs(1024) | p64(128) | dmask(128)
    tbl = nc.dram_tensor("tbl", [128, 2304], BF16, kind="ExternalInput")
    out = nc.dram_tensor("out", [S, HID], F32, kind="ExternalOutput")

    with tile.TileContext(nc) as tc, ExitStack() as top:
        # ---- pools (PSUM pools for the head loop are opened after the
        # streaming prologue releases its 8 banks) -----------------------
        const = top.enter_context(tc.tile_pool(name="const", bufs=1))
        xk_pool = top.enter_context(tc.tile_pool(name="xk", bufs=1))
        wkv_pool = top.enter_context(tc.tile_pool(name="wkv", bufs=1))
        kT_pool = top.enter_context(tc.tile_pool(name="kT", bufs=1))
        v_pool = top.enter_context(tc.tile_pool(name="v", bufs=1))
        ctxT_pool = top.enter_context(tc.tile_pool(name="ctxT", bufs=1))
        wq_pool = top.enter_context(tc.tile_pool(name="wq", bufs=2))
        qT_pool = top.enter_context(tc.tile_pool(name="qT", bufs=2))
        eS_pool = top.enter_context(tc.tile_pool(name="eS", bufs=2))
        dn_pool = top.enter_context(tc.tile_pool(name="dn", bufs=2))
        rc_pool = top.enter_context(tc.tile_pool(name="rc", bufs=2))
        tmp_pool = top.enter_context(tc.tile_pool(name="tmp", bufs=2))
        wo_pool = top.enter_context(tc.tile_pool(name="wo", bufs=1))
        out_pool = top.enter_context(tc.tile_pool(name="outp", bufs=3))

        # ---- input DMA: 3 HW queues (sync / gpsimd / scalar), biggest
        # early-need tensors split for parallelism ------------------------
        wk_sb = wkv_pool.tile([128, NKT * KVD], BF16, tag="wk")
        wv_sb = wkv_pool.tile([128, NKT * KVD], BF16, tag="wv")
        HK = NKT * KVD // 2
        nc.sync.dma_start(wk_sb[:, 0:HK], wk[:, 0:HK])
        nc.gpsimd.dma_start(wk_sb[:, HK:], wk[:, HK:])
        nc.scalar.dma_start(wv_sb[:], wv[:])

        # x^T in 8 pair-chunks alternating sync/gpsimd
        xkp = [xk_pool.tile([128, 2 * S], BF16, tag=f"xkp{g}", name=f"xkp{g}")
               for g in range(NKT // 2)]
        for g in range(NKT // 2):
            eng = nc.sync if g % 2 == 0 else nc.gpsimd
            eng.dma_start(xkp[g][:], xT[:, 2 * g * S:(2 * g + 2) * S])

        def xkc(kt, a, b):
            g, r = kt // 2, kt % 2
            return xkp[g][:, r * S + a:r * S + b]

        wq_sb = [wq_pool.tile([128, NKT * D], BF16, tag="wqh",
                              name=f"wqh{h}") for h in range(HPC)]
        nc.scalar.dma_start(wq_sb[0][:], wq[:, 0:NKT * D])
        tbl_sb = const.tile([128, 2304], BF16)
        nc.sync.dma_start(tbl_sb[:], tbl[:])
        nc.scalar.dma_start(wq_sb[1][:], wq[:, NKT * D:2 * NKT * D])
        cosT_sb = tbl_sb[:, 0:S]
        sinTs_sb = tbl_sb[:, S:2 * S]
        p64_sb = tbl_sb[:, 2 * S:2 * S + D]
        dmask_sb = tbl_sb[:, 2 * S + D:2 * S + 2 * D]

        ones_bf = const.tile([128, 1], BF16)
        nc.vector.memset(ones_bf[:], 1.0)

        ctxT = [ctxT_pool.tile([D, S], BF16, tag=f"ctxT{h}", name=f"ctxT{h}")
                for h in range(HPC)]
        kT = [kT_pool.tile([D, S], BF16, tag=f"kT{c}", name=f"kT{c}")
              for c in range(KVPC)]
        v_sb = v_pool.tile([128, NST * KVD], BF16)

        # ---- pass 1: stream kt chunks once -> all of K proj + V st0-3,
        # 8 concurrent PSUM accumulation groups chase the DMA -------------
        with ExitStack() as pro:
            psP = pro.enter_context(tc.tile_pool(name="psP", bufs=1,
                                                 space="PSUM"))
            kps = {}
            for c in range(KVPC):
                for sc in range(2):
                    kps[(c, sc)] = psP.tile([128, 512], F32, tag=f"kp{c}{sc}",
                                            name=f"kp{c}{sc}")
            vps = [psP.tile([128, 512], F32, tag=f"vp{st}", name=f"vp{st}")
                   for st in range(4)]
            for kt in range(NKT):
                for c in range(KVPC):
                    for sc in range(2):
                        nc.tensor.matmul(
                            kps[(c, sc)][:],
                            wk_sb[:, kt * KVD + c * D:kt * KVD + (c + 1) * D],
                            xkc(kt, sc * 512, (sc + 1) * 512),
                            start=(kt == 0), stop=(kt == NKT - 1))
                for st in range(4):
                    nc.tensor.matmul(
                        vps[st][:, 0:KVD], xkc(kt, st * 128, (st + 1) * 128),
                        wv_sb[:, kt * KVD:(kt + 1) * KVD],
                        start=(kt == 0), stop=(kt == NKT - 1))
            for c in range(KVPC):
                for sc in range(2):
                    nc.vector.tensor_copy(kT[c][:, sc * 512:(sc + 1) * 512],
                                          kps[(c, sc)][:])
            for st in range(4):
                nc.vector.tensor_copy(v_sb[:, st * KVD:(st + 1) * KVD],
                                      vps[st][:, 0:KVD])

        # head-loop PSUM pools (prologue banks now free)
        psQ = top.enter_context(tc.tile_pool(name="psQ", bufs=2, space="PSUM"))
        psS = top.enter_context(tc.tile_pool(name="psS", bufs=2, space="PSUM"))
        psC = top.enter_context(tc.tile_pool(name="psC", bufs=2, space="PSUM"))
        psD = top.enter_context(tc.tile_pool(name="psD", bufs=2, space="PSUM"))

        def rope_T(dst_sb, sc):
            """RoPE in T orientation on 512-col chunk sc of a [128, S] bf16
            tile, in place. P64 perm matmul + 3 DVE ops."""
            cs = slice(sc * 512, (sc + 1) * 512)
            rot = psD.tile([128, 512], F32, tag="ps_d")
            nc.tensor.matmul(rot[:], p64_sb, dst_sb[:, cs],
                             start=True, stop=True)
            tmp = tmp_pool.tile([128, 512], BF16, tag="ropetmp")
            nc.vector.tensor_mul(tmp[:], rot[:], sinTs_sb[:, cs])
            nc.vector.tensor_mul(dst_sb[:, cs], dst_sb[:, cs], cosT_sb[:, cs])
            nc.vector.tensor_add(dst_sb[:, cs], dst_sb[:, cs], tmp[:])

        # ---- pass 2: V st4-7 + K rope + Q0 proj -------------------------
        for st in range(4, NST):
            if st % 2 == 0:
                ps = psS.tile([128, 512], F32, tag="ps_s", name=f"vq{st}")
            else:
                ps = psC.tile([128, 512], F32, tag="ps_c", name=f"vq{st}")
            for kt in range(NKT):
                nc.tensor.matmul(
                    ps[:, 0:KVD], xkc(kt, st * 128, (st + 1) * 128),
                    wv_sb[:, kt * KVD:(kt + 1) * KVD],
                    start=(kt == 0), stop=(kt == NKT - 1))
            nc.vector.tensor_copy(v_sb[:, st * KVD:(st + 1) * KVD],
                                  ps[:, 0:KVD])
        for c in range(KVPC):
            for sc in range(2):
                rope_T(kT[c], sc)

        def q_proj(h):
            qT_h = qT_pool.tile([D, S], BF16, tag="qTh", name=f"qT{h}")
            for sc in range(2):
                ps = psQ.tile([128, 512], F32, tag="ps_q")
                for kt in range(NKT):
                    nc.tensor.matmul(
                        ps[:], wq_sb[h][:, kt * D:(kt + 1) * D],
                        xkc(kt, sc * 512, (sc + 1) * 512),
                        start=(kt == 0), stop=(kt == NKT - 1))
                # PSUM->SBUF cast on the scalar engine (DVE is busier)
                nc.scalar.copy(qT_h[:, sc * 512:(sc + 1) * 512], ps[:])
            for sc in range(2):
                rope_T(qT_h, sc)
            return qT_h

        qT_cur = q_proj(0)

        wo_sb = None
        for h in range(HPC):
            c = h // (HPC // KVPC)  # local kv head
            # -- scores + exp (ragged blocks per kt) ----------------------
            eS = eS_pool.tile([128, ES_W], BF16, tag="eS", name=f"eS{h}")
            for kt in range(NST):
                for (qlo, qhi) in _pieces(kt):
                    w = qhi - qlo
                    ps = psS.tile([128, 512], F32, tag="ps_s")
                    nc.tensor.matmul(
                        ps[:, 0:w], kT[c][:, kt * 128:(kt + 1) * 128],
                        qT_cur[:, qlo:qhi], start=True, stop=True)
                    nc.scalar.activation(
                        eS[:, ES_OFF[kt] + qlo - 128 * kt:
                            ES_OFF[kt] + qhi - 128 * kt],
                        ps[:, 0:w],
                        mybir.ActivationFunctionType.Exp, scale=SCALE)
                # mask the diagonal 128x128 block
                nc.vector.tensor_mul(
                    eS[:, ES_OFF[kt]:ES_OFF[kt] + 128],
                    eS[:, ES_OFF[kt]:ES_OFF[kt] + 128], dmask_sb)

            # -- denominator ragged pre-sum, issued BEFORE the next head's
            # rope so it trails exp(h) while the PE runs Q-proj.  Short j=0
            # chain on the slow gpsimd engine, long j=1 chain on DVE.
            dnp = dn_pool.tile([128, S], BF16, tag="dnp", name=f"dnp{h}")
            for j, eng in ((0, nc.gpsimd), (1, nc.vector)):
                first = True
                for kt in range(NST):
                    qlo = 128 * kt
                    lo = max(qlo, j * 512)
                    hi = min(S, (j + 1) * 512)
                    if lo >= hi:
                        continue
                    src = eS[:, ES_OFF[kt] + lo - qlo:ES_OFF[kt] + hi - qlo]
                    if first:
                        eng.tensor_copy(dnp[:, lo:hi], src)
                        first = False
                    else:
                        eng.tensor_add(dnp[:, lo:hi], dnp[:, lo:hi], src)

            # -- next head's Q proj fills the PE while exp(h) runs --------
            if h + 1 < HPC:
                if h + 2 < HPC:
                    nc.gpsimd.dma_start(
                        wq_sb[h + 2][:],
                        wq[:, (h + 2) * NKT * D:(h + 3) * NKT * D])
                qT_nxt = q_proj(h + 1)

            # -- denominator ones-matmul + reciprocal ---------------------
            rc = rc_pool.tile([1, S], F32, tag="rc", name=f"rc{h}")
            dps = [None, None]
            for j in range(2):
                dps[j] = psD.tile([128, 512], F32, tag="ps_d", name=f"dps{j}")
                nc.tensor.matmul(dps[j][:1, 0:512], ones_bf[:],
                                 dnp[:, j * 512:(j + 1) * 512],
                                 start=True, stop=True)
            for j in range(2):
                nc.vector.reciprocal_approx_fast(
                    rc[:1, j * 512:(j + 1) * 512], dps[j][:1, 0:512])

            # -- ctx matmul (ragged accumulate) + normalize ---------------
            pc = [None, None]
            for j in range(2):
                kts = [kt for kt in range(NST)
                       if max(128 * kt, j * 512) < (j + 1) * 512]
                pc[j] = psC.tile([128, 512], F32, tag="ps_c", name=f"pc{j}")
                for kt in kts:
                    qlo = 128 * kt
                    lo = max(qlo, j * 512)
                    hi = (j + 1) * 512
                    nc.tensor.matmul(
                        pc[j][:, lo - j * 512:hi - j * 512],
                        v_sb[:, kt * KVD + c * D:kt * KVD + (c + 1) * D],
                        eS[:, ES_OFF[kt] + lo - qlo:ES_OFF[kt] + hi - qlo],
                        start=(kt == kts[0]), stop=(kt == kts[-1]))
            # reciprocal broadcast on the (idle) gpsimd engine, off the PE
            for j in range(2):
                rb = tmp_pool.tile([128, 512], F32, tag="rbtmp")
                nc.gpsimd.partition_broadcast(
                    rb[:], rc[:1, j * 512:(j + 1) * 512])
                nc.vector.tensor_mul(
                    ctxT[h][:, j * 512:(j + 1) * 512], pc[j][:], rb[:])

            if h + 1 < HPC:
                qT_cur = qT_nxt
            if h == 3:  # wo arrives while attention still running
                wo_sb = wo_pool.tile([128, HPC * HID], BF16)
                nc.sync.dma_start(wo_sb[:], wo[:])

        # ---- O projection ----------------------------------------------
        for st in range(NST):
            for ec in range(HID // 512):
                if (st * 4 + ec) % 2:
                    po = psS.tile([128, 512], F32, tag="ps_s", name=f"po{st}_{ec}")
                else:
                    po = psQ.tile([128, 512], F32, tag="ps_q", name=f"po{st}_{ec}")
                for h in range(HPC):
                    nc.tensor.matmul(
                        po[:], ctxT[h][:, st * 128:(st + 1) * 128],
                        wo_sb[:, h * HID + ec * 512:h * HID + (ec + 1) * 512],
                        start=(h == 0), stop=(h == HPC - 1))
                ot = out_pool.tile([128, 512], F32, tag="ot")
                if (st * 4 + ec) % 2:
                    nc.scalar.copy(ot[:], po[:])
                    nc.scalar.dma_start(
                        out[st * 128:(st + 1) * 128,
                            ec * 512:(ec + 1) * 512], ot[:])
                else:
                    nc.vector.tensor_copy(ot[:], po[:])
                    nc.sync.dma_start(
                        out[st * 128:(st + 1) * 128,
                            ec * 512:(ec + 1) * 512], ot[:])
    nc.finalize()
    return nc


def host_prep(hidden_states, Wq, Wk, Wv, Wo):
    """Pre-transpose/cast/relayout all inputs on the host (bf16)."""
    bf = ml_dtypes.bfloat16
    xTs = []
    for b in range(B):
        t = hidden_states[b].T.reshape(NKT, 128, S).transpose(1, 0, 2)
        xTs.append(np.ascontiguousarray(t.astype(bf)).reshape(128, NKT * S))
    halves = []
    for hf in range(2):
        wqh = Wq[:, 1024 * hf:1024 * (hf + 1)].reshape(NKT, 128, HPC, D)
        wqh = np.ascontiguousarray(
            wqh.transpose(1, 2, 0, 3).astype(bf)).reshape(128, HPC * NKT * D)
        wkh = Wk[:, KVD * hf:KVD * (hf + 1)].reshape(NKT, 128, KVD)
        wkh = np.ascontiguousarray(
            wkh.transpose(1, 0, 2).astype(bf)).reshape(128, NKT * KVD)
        wvh = Wv[:, KVD * hf:KVD * (hf + 1)].reshape(NKT, 128, KVD)
        wvh = np.ascontiguousarray(
            wvh.transpose(1, 0, 2).astype(bf)).reshape(128, NKT * KVD)
        woh = Wo[1024 * hf:1024 * (hf + 1), :].reshape(HPC, 128, HID)
        woh = np.ascontiguousarray(
            woh.transpose(1, 0, 2).astype(bf)).reshape(128, HPC * HID)
        halves.append((wqh, wkh, wvh, woh))

    inv_freq = 1.0 / (10000.0 ** (np.arange(0, D, 2, dtype=np.float64) / D))
    t = np.arange(S, dtype=np.float64)
    freqs = np.outer(t, inv_freq)
    emb = np.concatenate([freqs, freqs], -1)
    cosT = np.cos(emb).T
    sinTs_f = np.sin(emb).T.copy()
    sinTs_f[:64] *= -1.0
    p64 = np.zeros((D, D), dtype=np.float64)
    for d in range(D):
        p64[d, (d + 64) % D] = 1.0
    dmask = np.triu(np.ones((128, 128), dtype=np.float64))
    tbl = np.ascontiguousarray(
        np.concatenate([cosT, sinTs_f, p64, dmask], axis=1)).astype(bf)
    return xTs, halves, tbl


_CACHE = {}


def kernel(hidden_states, Wq, Wk, Wv, Wo, _trace=False, _tmpdir=None):
    hidden_states = np.ascontiguousarray(hidden_states, dtype=np.float32)
    Wq = np.ascontiguousarray(Wq, dtype=np.float32)
    Wk = np.ascontiguousarray(Wk, dtype=np.float32)
    Wv = np.ascontiguousarray(Wv, dtype=np.float32)
    Wo = np.ascontiguousarray(Wo, dtype=np.float32)

    if "nc" not in _CACHE:
        _CACHE["nc"] = build_kernel()
    nc = _CACHE["nc"]
    xTs, halves, tbl = host_prep(hidden_states, Wq, Wk, Wv, Wo)

    in_maps = []
    for cid in range(8):
        b, hf = cid // 2, cid % 2
        wqh, wkh, wvh, woh = halves[hf]
        in_maps.append({
            "xT": xTs[b], "wq": wqh, "wk": wkh, "wv": wvh, "wo": woh,
            "tbl": tbl,
        })
    res = run_bass_kernel_spmd(nc, in_maps, list(range(8)),
                               trace=_trace, tmpdir=_tmpdir)
    out = np.zeros((B, S, HID), dtype=np.float32)
    for cid in range(8):
        out[cid // 2] += res.results[cid]["out"]
    if _trace:
        return out, res
    return out
